# revision 9
# baseline (speedup 1.0000x reference)
"""Anisotropic diffusion step on 8 TRN2 NeuronCores (v2).

Problem: x_new = x + sigmoid(rate) * cond * lap * (1 - mask)
  grad = Sobel(x); gm = sqrt(gx^2+gy^2+eps)
  cond = sigmoid(W2 @ relu(W1 @ [gm; x] + b1) + b2)   (1x1 convs)
  lap  = Laplacian(x)

Sharding: data-parallel over batch. B=8, one image (64,256,256) per core.

Per-core layout: partitions 0-63 = 64 channels of image-half A (rows
0..127), partitions 64-127 = channels of half B (rows 128..255).
x is stored per partition as CONTIGUOUS bf16 rows -1..128 (130 rows of
256, row stride 256) so the HBM load uses big contiguous descriptors.
An fp8 copy of x streams in per 8-row block for the Sobel-y DoubleRow
matmuls.  Column-shift (c +/- 1) ops read the contiguous rows with
wraparound at row ends; the two border columns are patched afterwards
by tiny strided fix ops.

Engine split per 8-row block (4 chunks of 512):
  TensorE : 3 Sobel-y DR matmuls/chunk, W1A+W1B, W2D, mask LP bcast
  ScalarE : sqrt (block-wide), relu, sigmoid (per chunk)
  DVE     : s_v=x[r-1]+x[r+1], P=2x+s_v, gx=P>>1 - P<<1, SQSUM/chunk,
            t3=s_v+t2, lap=t3-4x, CL=lap*CDM, xo=x+CL (block-wide)
  GpSimd  : t2 = x<<1 + x>>1, CDM = CD*mps (PSUM read)
sqrt vs relu/sigmoid phases are split per superblock so the ScalarE
activation-table set is switched only 4x.  rate*(1-mask) and all weight
transforms are precomputed on the host.
"""

import numpy as np
from contextlib import ExitStack

C = 64
H = 256
W = 256
HID = 64
HH = 128          # rows per half
NR = 8            # rows per block (per half)
NBLK = H // 2 // NR   # 16 blocks total
NSB = 2           # super blocks
BLK_PER_SB = NBLK // NSB
XR = HH + 2       # rows per half incl halos (130), contiguous stride W
SBR = HH // NSB   # rows per superblock (64)
EPS = 1e-8

_CACHE = {}


def _register_custom_ops():
    from concourse import dve_ops as DO
    if any(o.name == "ANT_SQSUM" for o in DO.OPS):
        return
    from concourse.dve_spec import Spec, Src0, Src1, C2, sq, lower, _has_src1
    from concourse.dve_uop import DveOpSpec
    body = sq(Src0) + sq(Src1) + C2
    spec = Spec(body=body,
                reference=lambda in0, in1, s0, s1, imm2: in0 * in0 + in1 * in1 + imm2)
    shas = {}
    for ver in ("v3", "v4"):
        tmp = DveOpSpec(name="ANT_SQSUM", uops=lower(spec, ver=ver),
                        rd1_en=_has_src1(spec))
        shas[ver] = tmp.sha(ver)
    op = DO.DveOp("ANT_SQSUM", spec, subdim=False, uops_sha=shas)
    DO.OPS.append(op)
    DO.CUSTOM_DVE_SPECS[op.name] = op.spec
    DO._SUB_OPCODE_FOR_NAME[op.name] = DO._CUSTOM_DVE_ROW_BASE + len(DO.OPS) - 1


def _build():
    import concourse.bass as bass
    import concourse.bacc as bacc
    import concourse.tile as tile
    from concourse import mybir
    import bass_rust as _br
    ALU = mybir.AluOpType

    f32 = mybir.dt.float32
    bf16 = mybir.dt.bfloat16
    f8 = mybir.dt.float8e4
    AF = mybir.ActivationFunctionType

    _register_custom_ops()
    from concourse.dve_ops import OPS as _DVE_OPS
    SQSUM = [o for o in _DVE_OPS if o.name == "ANT_SQSUM"][0]

    nc = bacc.Bacc()

    x_ext = nc.declare_dram_parameter("x", [C, H, W], bf16, isOutput=False)
    xf8_ext = nc.declare_dram_parameter("xf8", [C, H, W], f8, isOutput=False)
    mp_ext = nc.declare_dram_parameter("mp", [2, HH * W], bf16, isOutput=False)
    w1at_ext = nc.declare_dram_parameter("w1at", [128, 128], bf16, isOutput=False)
    w1bt_ext = nc.declare_dram_parameter("w1bt", [128, 128], bf16, isOutput=False)
    w2d_ext = nc.declare_dram_parameter("w2d", [128, 128], bf16, isOutput=False)
    b1d_ext = nc.declare_dram_parameter("b1d", [128, 1], f32, isOutput=False)
    b2d_ext = nc.declare_dram_parameter("b2d", [128, 1], f32, isOutput=False)
    edr1_ext = nc.declare_dram_parameter("edr1", [128, 256], f8, isOutput=False)
    edr2_ext = nc.declare_dram_parameter("edr2", [128, 256], f8, isOutput=False)
    out_ext = nc.declare_dram_parameter("out", [C, H, W], bf16, isOutput=True)

    ctx = ExitStack()
    with tile.TileContext(nc) as tc, ctx:
        persist = ctx.enter_context(tc.tile_pool(name="persist", bufs=1))
        wpool = ctx.enter_context(tc.tile_pool(name="wts", bufs=1))
        scr = ctx.enter_context(tc.tile_pool(name="scr", bufs=2))
        psum = ctx.enter_context(tc.tile_pool(name="ps", bufs=2, space="PSUM"))

        # ---------------- persistent tiles ----------------
        XB = persist.tile([128, XR * W], bf16, tag="XB")     # contiguous x
        SV = persist.tile([128, SBR * W], bf16, tag="SV")    # x[r-1]+x[r+1], per SB
        GM = persist.tile([128, SBR * W], bf16, tag="GM")    # grad mag, per SB

        xb = XB[:].rearrange("p (r c) -> p r c", r=XR)   # tile row t = img row t-1
        sv3 = SV[:].rearrange("p (r c) -> p r c", r=SBR)
        gm3 = GM[:].rearrange("p (r c) -> p r c", r=SBR)

        # ---------------- weights ----------------
        W1A = wpool.tile([128, 128], bf16, tag="W1A")
        W1B = wpool.tile([128, 128], bf16, tag="W1B")
        W2D = wpool.tile([128, 128], bf16, tag="W2D")
        B1 = wpool.tile([128, 1], f32, tag="B1")
        B2 = wpool.tile([128, 1], f32, tag="B2")
        EDR1 = wpool.tile([128, 256], f8, tag="EDR1")
        EDR2 = wpool.tile([128, 256], f8, tag="EDR2")
        ZB = wpool.tile([128, 1], f32, tag="ZB")
        nc.vector.memset(ZB[:, :], 0.0)

        nc.sync.dma_start(out=W1A[:, :], in_=w1at_ext[:, :])
        nc.sync.dma_start(out=W1B[:, :], in_=w1bt_ext[:, :])
        nc.sync.dma_start(out=W2D[:, :], in_=w2d_ext[:, :])
        nc.sync.dma_start(out=B1[:, :], in_=b1d_ext[:, :])
        nc.sync.dma_start(out=B2[:, :], in_=b2d_ext[:, :])
        nc.sync.dma_start(out=EDR1[:, :], in_=edr1_ext[:, :])
        nc.sync.dma_start(out=EDR2[:, :], in_=edr2_ext[:, :])

        tc.strict_bb_all_engine_barrier()

        # ---------------- x load: halos + big contiguous groups ----------
        nc.vector.memset(xb[0:64, 0, :], 0.0)            # half A img row -1
        nc.vector.memset(xb[64:128, XR - 1, :], 0.0)     # half B img row 256
        nc.sync.dma_start(out=xb[0:64, XR - 1, :], in_=x_ext[:, HH, :])
        nc.sync.dma_start(out=xb[64:128, 0, :], in_=x_ext[:, HH - 1, :])
        GRP = 16   # rows per load group
        for gi in range(HH // GRP):
            r = gi * GRP
            nc.sync.dma_start(out=xb[0:64, r + 1:r + 1 + GRP, :],
                              in_=x_ext[:, r:r + GRP, :])
            nc.sync.dma_start(out=xb[64:128, r + 1:r + 1 + GRP, :],
                              in_=x_ext[:, HH + r:HH + r + GRP, :])

        # ---------------- helper views ----------------
        def flat_rows(base_tile, free_elems, row0, nrows, dc=0):
            """[128, nrows, 256] view with arbitrary element offset."""
            c = base_tile[:].copy()
            c.ap = _br.VecI64Pair([(free_elems, 128), (W, nrows), (1, W)])
            c.offset = row0 * W + dc
            return c

        def view4(base_tile, free_elems, pairs, offset):
            c = base_tile[:].copy()
            c.ap = _br.VecI64Pair([(free_elems, 128)] + list(pairs))
            c.offset = offset
            return c

        def xf8_pair_view(tl, cidx, dc):
            """(128, 2, 2, 256) DR pair view on the rolling fp8 tile.

            Data rows live at tile rows 1..10 (guard rows 0 and 11 keep the
            +/-1 column shifts in-bounds and offsets positive)."""
            c = tl[:].copy()
            c.ap = _br.VecI64Pair([(12 * W, 128), (2 * W, 2), (W, 2), (1, W)])
            c.offset = (1 + 2 * cidx) * W + dc
            return c

        def load_xf8(g):
            """fp8 rows img r0-1..r0+8 of each half into rolling tile rows 1..10."""
            tl = scr.tile([128, 12 * W], f8, tag="xf8")
            t3v = tl[:].rearrange("p (r c) -> p r c", r=12)
            r0 = g * NR
            if g == 0:
                nc.vector.memset(t3v[0:64, 1, :], 0.0)
                nc.sync.dma_start(out=t3v[0:64, 2:11, :], in_=xf8_ext[:, 0:9, :])
                nc.sync.dma_start(out=t3v[64:128, 1:11, :],
                                  in_=xf8_ext[:, HH - 1:HH + 9, :])
            elif g == NBLK - 1:
                nc.sync.dma_start(out=t3v[0:64, 1:11, :],
                                  in_=xf8_ext[:, r0 - 1:r0 + 9, :])
                nc.sync.dma_start(out=t3v[64:128, 1:10, :],
                                  in_=xf8_ext[:, HH + r0 - 1:HH + r0 + 8, :])
                nc.vector.memset(t3v[64:128, 10, :], 0.0)
            else:
                nc.sync.dma_start(out=t3v[0:64, 1:11, :],
                                  in_=xf8_ext[:, r0 - 1:r0 + 9, :])
                nc.sync.dma_start(out=t3v[64:128, 1:11, :],
                                  in_=xf8_ext[:, HH + r0 - 1:HH + r0 + 9, :])
            return tl

        XBF = XR * W   # free elems of XB

        def stencil_block(g, sb, xf8t):
            """phase 1: gm rows [g*NR, g*NR+NR) of each half."""
            r0 = g * NR
            lr0 = r0 - sb * SBR
            # s_v = x[r-1] + x[r+1]  (into SB-persistent SV)
            nc.vector.tensor_tensor(
                sv3[:, lr0:lr0 + NR, :],
                xb[:, r0:r0 + NR, :], xb[:, r0 + 2:r0 + 2 + NR, :], ALU.add)
            # P = 2x + s_v on padded width 258
            P = scr.tile([128, NR * (W + 2)], bf16, tag="pp")
            p3 = P[:].rearrange("p (r c) -> p r c", r=NR)
            nc.vector.memset(view4(P, NR * (W + 2), [(W + 2, NR), (W + 1, 2)], 0), 0.0)
            nc.vector.scalar_tensor_tensor(
                p3[:, :, 1:W + 1], xb[:, r0 + 1:r0 + 1 + NR, :], 2.0,
                sv3[:, lr0:lr0 + NR, :], ALU.mult, ALU.add)
            # gx = P[c+1] - P[c-1]
            GX = scr.tile([128, NR * W], bf16, tag="gx")
            nc.vector.tensor_tensor(
                GX[:].rearrange("p (r c) -> p r c", r=NR),
                p3[:, :, 2:W + 2], p3[:, :, 0:W], ALU.subtract)
            # gy (3 DR matmuls per chunk) then g2 = gx^2 + gy^2 + eps
            G2 = scr.tile([128, NR * W], bf16, tag="g2")
            for cidx in range(NR // 2):
                gyp = psum.tile([128, 512], mybir.dt.float32, tag="gyp", bufs=4)
                for ti, (dc, edr) in enumerate([(-1, EDR1), (0, EDR2), (1, EDR1)]):
                    nc.tensor.matmul(
                        gyp[:, :],
                        edr[:].rearrange("p (a m) -> p a m", a=2),
                        xf8_pair_view(xf8t, cidx, dc),
                        start=(ti == 0), stop=(ti == 2),
                        perf_mode=mybir.MatmulPerfMode.DoubleRow)
                nc.vector._custom_dve(
                    SQSUM, out=G2[:, cidx * 512:(cidx + 1) * 512],
                    in0=gyp[:, :], in1=GX[:, cidx * 512:(cidx + 1) * 512],
                    imm2=EPS)
            # border fix: gy at c in {0,255} was computed with wraparound.
            # QA = x[r+1] - x[r-1] at cols {0,1,254,255}; gyb = 2*QA@{0,3}+QA@{1,2}
            QA = scr.tile([128, 4 * NR], bf16, tag="qa")
            nc.vector.tensor_tensor(
                view4(QA, 4 * NR, [(4, NR), (2, 2), (1, 2)], 0),
                view4(XB, XBF, [(W, NR), (254, 2), (1, 2)], (r0 + 2) * W),
                view4(XB, XBF, [(W, NR), (254, 2), (1, 2)], r0 * W),
                ALU.subtract)
            GYB = scr.tile([128, 2 * NR], bf16, tag="gyb")
            nc.vector.scalar_tensor_tensor(
                view4(GYB, 2 * NR, [(2, NR), (1, 2)], 0),
                view4(QA, 4 * NR, [(4, NR), (3, 2)], 0), 2.0,
                view4(QA, 4 * NR, [(4, NR), (1, 2)], 1),
                ALU.mult, ALU.add)
            nc.vector._custom_dve(
                SQSUM,
                out=view4(G2, NR * W, [(W, NR), (W - 1, 2)], 0),
                in0=view4(GX, NR * W, [(W, NR), (W - 1, 2)], 0),
                in1=GYB[:, 0:2 * NR],
                imm2=EPS)
            # gm = sqrt(g2)
            nc.scalar.activation(gm3[:, lr0:lr0 + NR, :],
                                 G2[:].rearrange("p (r c) -> p r c", r=NR),
                                 AF.Sqrt, bias=ZB[:, :])

        def mpb_bcast_view(t, h):
            """64x partition-replicated view of MPB partition h."""
            c = t[h:h + 1, :].copy()
            c.ap = _br.VecI64Pair([(NR * W, 1), (0, 64), (1, NR * W)])
            c.offset = h * NR * W
            return c

        def phase2_block(g, sb):
            """phase 2: convs + laplacian + update for block g."""
            r0 = g * NR
            lr0 = r0 - sb * SBR
            MPB = scr.tile([2, NR * W], bf16, tag="mpb")
            nc.sync.dma_start(out=MPB[:, :], in_=mp_ext[:, r0 * W:(r0 + NR) * W])
            # broadcast mp to all 128 partitions via SBUF->SBUF DMA
            MPS = scr.tile([128, NR * W], bf16, tag="mps")
            nc.sync.dma_start(out=MPS[0:64, :], in_=mpb_bcast_view(MPB, 0))
            nc.sync.dma_start(out=MPS[64:128, :], in_=mpb_bcast_view(MPB, 1))
            # t2 = x[c-1] + x[c+1] on GpSimd (wraparound, fixed below)
            T2 = scr.tile([128, NR * W], bf16, tag="sa")
            nc.gpsimd.tensor_tensor(
                T2[:].rearrange("p (r c) -> p r c", r=NR),
                flat_rows(XB, XBF, r0 + 1, NR, -1),
                flat_rows(XB, XBF, r0 + 1, NR, +1), ALU.add)
            nc.vector.tensor_scalar_mul(
                view4(T2, NR * W, [(W, NR), (W - 1, 2)], 0),
                view4(XB, XBF, [(W, NR), (W - 3, 2)], (r0 + 1) * W + 1), 1.0)
            # t3 = s_v + t2 ; lap = t3 - 4x
            T3 = scr.tile([128, NR * W], bf16, tag="sb")
            nc.gpsimd.tensor_tensor(
                T3[:, :], SV[:, lr0 * W:(lr0 + NR) * W], T2[:, :], ALU.add)
            LAP = scr.tile([128, NR * W], bf16, tag="pp")
            nc.vector.scalar_tensor_tensor(
                LAP[:, 0:NR * W], XB[:, (r0 + 1) * W:(r0 + 1 + NR) * W], -4.0,
                T3[:, :], ALU.mult, ALU.add)
            CD = scr.tile([128, NR * W], bf16, tag="cd")
            for cidx in range(NR // 2):
                la = lr0 + 2 * cidx
                ra = r0 + 2 * cidx
                hps = psum.tile([128, 512], mybir.dt.float32, tag="hps")
                nc.tensor.matmul(hps[:, :], W1A[:, :], gm3[:, la:la + 2, :],
                                 start=True, stop=False)
                nc.tensor.matmul(hps[:, :], W1B[:, :], xb[:, ra + 1:ra + 3, :],
                                 start=False, stop=True)
                HR = scr.tile([128, 512], bf16, tag="hr")
                nc.scalar.activation(HR[:, :], hps[:, :], AF.Relu, bias=B1[:, :])
                zps = psum.tile([128, 512], mybir.dt.float32, tag="zps")
                nc.tensor.matmul(zps[:, :], W2D[:, :], HR[:, :], start=True, stop=True)
                nc.scalar.activation(CD[:, cidx * 512:(cidx + 1) * 512],
                                     zps[:, :], AF.Sigmoid, bias=B2[:, :])
            # CDM = cond * mp ; CL = CDM * lap ; xo = x + CL
            CDM = scr.tile([128, NR * W], bf16, tag="g2")
            nc.vector.tensor_tensor(CDM[:, :], CD[:, :], MPS[:, :], ALU.mult)
            CL = scr.tile([128, NR * W], bf16, tag="sb")
            nc.vector.tensor_tensor(CL[:, :], LAP[:, 0:NR * W], CDM[:, :], ALU.mult)
            XO = scr.tile([128, NR * W], bf16, tag="sa")
            nc.gpsimd.tensor_tensor(
                XO[:, :], XB[:, (r0 + 1) * W:(r0 + 1 + NR) * W], CL[:, :],
                ALU.add)
            xo3 = XO[:].rearrange("p (r c) -> p r c", r=NR)
            nc.sync.dma_start(out=out_ext[:, r0:r0 + NR, :], in_=xo3[0:64])
            nc.sync.dma_start(out=out_ext[:, HH + r0:HH + r0 + NR, :], in_=xo3[64:128])

        # ================= main schedule =================
        xf8_tiles = {0: load_xf8(0)}
        for sb in range(NSB):
            g0 = sb * BLK_PER_SB
            for g in range(g0, g0 + BLK_PER_SB):
                if g + 1 < NBLK and g + 1 not in xf8_tiles:
                    xf8_tiles[g + 1] = load_xf8(g + 1)
                stencil_block(g, sb, xf8_tiles.pop(g))
            tc.no_sync_barrier()
            for g in range(g0, g0 + BLK_PER_SB):
                phase2_block(g, sb)
            tc.no_sync_barrier()

    nc.compile()
    return nc


def _get_nc():
    if "nc" not in _CACHE:
        _CACHE["nc"] = _build()
    return _CACHE["nc"]


def _run(inputs, trace=False):
    from concourse.bass_utils import run_bass_kernel_spmd
    from concourse import mybir as _mb0
    import ml_dtypes

    nc = _get_nc()
    bf = ml_dtypes.bfloat16
    f8np = _mb0.dt.np(_mb0.dt.float8e4)
    x = np.asarray(inputs["x"], dtype=np.float32)
    mask = np.asarray(inputs["mask"], dtype=np.float32)
    w1 = np.asarray(inputs["w1"], dtype=np.float32)
    b1 = np.asarray(inputs["b1"], dtype=np.float32).reshape(HID, 1)
    w2 = np.asarray(inputs["w2"], dtype=np.float32)
    b2 = np.asarray(inputs["b2"], dtype=np.float32).reshape(C, 1)
    dr = float(np.asarray(inputs["diffusion_rate"], dtype=np.float32).reshape(()))

    w1at = np.zeros((128, 128), dtype=np.float32)
    w1at[0:C, 0:HID] = w1[:, 0:C].T
    w1at[C:128, HID:128] = w1[:, 0:C].T
    w1at = w1at.astype(bf)
    w1bt = np.zeros((128, 128), dtype=np.float32)
    w1bt[0:C, 0:HID] = w1[:, C:2 * C].T
    w1bt[C:128, HID:128] = w1[:, C:2 * C].T
    w1bt = w1bt.astype(bf)
    w2d = np.zeros((128, 128), dtype=np.float32)
    w2d[0:HID, 0:C] = w2.T
    w2d[HID:128, C:128] = w2.T
    w2d = w2d.astype(bf)
    b1d = np.concatenate([b1, b1], axis=0)
    b2d = np.concatenate([b2, b2], axis=0)
    eye = np.eye(128, dtype=np.float32)

    def _dr(w0, w1_):
        a = np.zeros((128, 2, 128), dtype=np.float32)
        a[:, 0, :] = eye * w0
        a[:, 1, :] = eye * w1_
        return np.ascontiguousarray(a.reshape(128, 256)).astype(f8np)
    edr1 = _dr(-1.0, 1.0)
    edr2 = _dr(-2.0, 2.0)

    rate = 1.0 / (1.0 + np.exp(-dr))
    xbf = np.ascontiguousarray(x.astype(bf))
    xf8a = np.ascontiguousarray(x.astype(f8np))
    B = x.shape[0]
    in_maps = []
    for b in range(B):
        mp = (rate * (1.0 - mask[b, 0])).astype(np.float32)
        mp2 = np.stack([mp[0:HH].reshape(-1), mp[HH:H].reshape(-1)], axis=0)
        in_maps.append({
            "x": xbf[b],
            "xf8": xf8a[b],
            "mp": np.ascontiguousarray(mp2.astype(bf)),
            "w1at": w1at, "w1bt": w1bt, "w2d": w2d,
            "b1d": b1d, "b2d": b2d,
            "edr1": edr1, "edr2": edr2,
        })
    res = run_bass_kernel_spmd(nc, in_maps, core_ids=list(range(8)), trace=trace)
    out = np.stack([np.asarray(res.results[i]["out"]).astype(np.float32)
                    for i in range(B)], axis=0)
    return out, res.exec_time_ns


def kernel(**inputs):
    return _run(inputs, trace=False)[0]
